# revision 25
# baseline (speedup 1.0000x reference)
"""DLRM DotInteractionArch kernel for 8x Trainium2 NeuronCores.

Problem: B=16384, 26 sparse embeddings + 1 dense feature, D=128.
  combined[b] = concat(dense[b], emb[b])           # [27, 128]
  G[b] = combined[b] @ combined[b].T               # [27, 27]
  out[b] = concat(dense[b], triu(G[b], k=1).flat)  # [479]

V11 strategy (pure data parallel, 2048 samples/core, 16 rounds x 128 samples):
  - The fp32->bf16 cast happens ON THE HOST (numpy/ml_dtypes) inside
    kernel(); the device receives bf16 embeddings + bf16 dense (for the
    interaction matmuls) and an fp32 dense copy (for the exact passthrough
    columns). This halves HBM load bytes and avoids the SWDGE cast path,
    which converts at only ~8GB/s per DMA engine.
  - Loads SAMPLE-MAJOR: one SWDGE DMA per round per tensor
    ([128 b-partitions, 26*128] bf16, 128 fat descriptors, ~6.7KB each).
  - PE transpose pass: 27 bf16 is_transpose matmuls [128b,128d] -> PSUM
    bf16. Evac copy reads PT [d, j, b] with a rearranged (strided) PSUM AP
    and writes CT [128 d, 128 b, 32 f-slot] bf16 contiguously.
    (Reordered DVE *reads* are fine; reordered bf16 *writes* crash.)
  - G-pass: per-sample col-tiled matmuls with CONTIGUOUS operands
    CT[:, b, 0:27] (tile_position (0,32s)) -> PSG [32s+f, q, g'] fp32.
    Evacuated by the Scalar engine (ACT) into 4-round slab staging.
  - Stores: run-f descriptors are partition-pinned to SDMA engine pair
    (f//4)*2,(+1); HWDGE-ring descriptors additionally pin to engines 0-3.
    So runs f=0..7 (engines 0-3 either way) go per-round on sync/scalar
    HWDGE (many small concurrent transfers pipeline best), and runs
    f=8..25 go on gpsimd SWDGE in 4-round slabs (descriptors spread to
    engines 4-13; Q7 emission amortized 4x), emitted spread over the
    rounds after each slab.
  - Dense passthrough: one fp32 HBM->HBM SWDGE DMA.
  - gpsimd slab stores use single_packet=True: their ~512 scattered
    54B descriptors pack into single packets, cutting per-packet
    m2s<->s2m context-switch overhead on the SDMA engines (-6% wall;
    on the HWDGE per-round stores the same flag HURTS - they rely on
    multi-engine spread).

NOTE: stride-partition APs (ST[f::32]) are invisible to the Tile
shadow-memory dependency tracker, so RAW/WAR edges around the store DMAs
are wired explicitly with add_dep_helper.

Sample mapping per core: b = r*128 + g*4 + s  (r: round, g = h*16+q, s: 0..3)
"""

import numpy as np

B_FULL = 16384
N_CORES = 8
BC = B_FULL // N_CORES  # 2048 samples per core
F = 27                  # 1 dense + 26 sparse features
D = 128
NSPARSE = 26
SPR = 128               # samples per round
PAIRS = F * (F - 1) // 2  # 351
OUTC = D + PAIRS          # 479

_CACHE = {}


def _triu_offsets():
    off = [D]
    for f in range(F - 1):
        off.append(off[-1] + (NSPARSE - f))
    return off


def _build_nc(bc: int = BC):
    from contextlib import ExitStack

    import concourse.bacc as bacc
    import concourse.tile as tile
    from concourse import mybir
    from concourse.masks import make_identity
    from concourse.tile_rust import add_dep_helper

    BF = mybir.dt.bfloat16
    F32 = mybir.dt.float32
    R = bc // SPR  # 16 rounds
    SLAB = 4       # staging slab size in rounds
    ST_BUFS = 3
    HW_MAX_F = 0   # all runs on single-packet gpsimd slabs

    nc = bacc.Bacc("TRN2", target_bir_lowering=False, debug=False)
    den_f = nc.dram_tensor("dense_f32", [bc, D], F32, kind="ExternalInput")
    den = nc.dram_tensor("dense_bf16", [bc, D], BF, kind="ExternalInput")
    emb = nc.dram_tensor("emb_bf16", [bc, NSPARSE, D], BF, kind="ExternalInput")
    out = nc.dram_tensor("out", [bc, OUTC], F32, kind="ExternalOutput")

    off = _triu_offsets()

    emb_v = emb.ap().rearrange("(r p) j d -> r p j d", p=SPR)
    den_v = den.ap().rearrange("(r p) d -> r p d", p=SPR)
    out_v = out.ap().rearrange("(r h q s) c -> r s h q c", h=2, q=16, s=4)

    with tile.TileContext(nc) as tc, ExitStack() as ctx:
        const = ctx.enter_context(tc.tile_pool(name="const", bufs=1))
        xep = ctx.enter_context(tc.tile_pool(name="xe", bufs=3))
        xdp = ctx.enter_context(tc.tile_pool(name="xd", bufs=3))
        ctp = ctx.enter_context(tc.tile_pool(name="ct", bufs=3))
        stp = ctx.enter_context(tc.tile_pool(name="st", bufs=ST_BUFS))
        ptp = ctx.enter_context(tc.tile_pool(name="pt", bufs=3, space="PSUM"))
        psgp = ctx.enter_context(tc.tile_pool(name="psg", bufs=4, space="PSUM"))

        ident = const.tile([128, 128], BF)
        make_identity(nc, ident)

        # dense passthrough columns: one fp32 HBM->HBM DMA
        nc.gpsimd.dma_start(out=out.ap()[:, 0:D], in_=den_f.ap()[:, :])

        st_copies = {}   # slab -> [evac insts]
        slab_dmas = {}   # slab -> [store insts] (for WAR on slot reuse)
        pending = {}     # round -> [(slab, r_lo, r_hi, f, eng)]
        ST_tiles = {}    # slab -> tile

        def emit_store(slab, r_lo, r_hi, f, eng):
            n = NSPARSE - f
            src = ST_tiles[slab][f::32, r_lo - slab * SLAB:r_hi - slab * SLAB,
                                 :, :, f + 1:F]
            dst = (
                out.ap()[r_lo * SPR:r_hi * SPR, off[f]:off[f] + n]
                .rearrange("(r h q s) c -> s r h q c", h=2, q=16, s=4)
            )
            d = eng.dma_start(out=dst, in_=src,
                              single_packet=(eng is nc.gpsimd))
            slab_dmas.setdefault(slab, []).append(d)
            for cpy in st_copies[slab]:
                add_dep_helper(d.ins, cpy.ins, reason="triu DMA RAW on ST")

        for r in range(R):
            slab = r // SLAB

            # ---- stores scheduled for this round ----
            for args in pending.pop(r, []):
                emit_store(*args)

            # ---- loads: sample-major bf16 on the sync HWDGE ring (keeps them
            # out of the gpsimd SWDGE ring, where slab-store descriptor bursts
            # would block them FIFO-style) ----
            XE = xep.tile([128, NSPARSE, D], BF)
            XD = xdp.tile([128, D], BF)
            nc.sync.dma_start(out=XE[:], in_=emb_v[r])
            nc.sync.dma_start(out=XD[:], in_=den_v[r])

            # ---- T-pass: 27 bf16 transposes [128 b, 128 d] -> PT [128 d, j, 128 b]
            CT = ctp.tile([128, 128, 32], BF)  # [d, b, f-slot]
            for t in range(7):
                js = list(range(4 * t, min(4 * t + 4, F)))
                k = len(js)
                PT = ptp.tile([128, 4, 128], BF)
                for i, j in enumerate(js):
                    src = XD[:] if j == 0 else XE[:, j - 1]
                    nc.tensor.transpose(PT[:, i], src, ident[:])
                pin = PT[:, 0:k].rearrange("d j b -> d b j")
                nc.vector.tensor_copy(CT[:, :, js[0]:js[0] + k], pin)

            # ---- G-pass: per-sample col-tiled matmuls -> PSG [32s+f, q, g']
            if slab not in ST_tiles:
                ST_tiles[slab] = stp.tile([128, SLAB, 2, 16, 32], F32, name="ST")
                st_copies[slab] = []
            ST = ST_tiles[slab]
            rs = r % SLAB
            for h in range(2):
                PSG = psgp.tile([128, 16, 32], F32)
                for q in range(16):
                    g = h * 16 + q
                    for s in range(4):
                        c = CT[:, 4 * g + s, 0:F]  # [128 d, 27 f] contiguous
                        nc.tensor.matmul(
                            PSG[32 * s:32 * s + F, q, 0:F],
                            c,
                            c,
                            start=True,
                            stop=True,
                            tile_position=(0, 32 * s),
                        )
                cpy = nc.scalar.copy(ST[:, rs, h], PSG[:])
                st_copies[slab].append(cpy)
                # WAR: this evac reuses the slot read by slab-ST_BUFS stores
                for d in slab_dmas.get(slab - ST_BUFS, []):
                    add_dep_helper(cpy.ins, d.ins, reason="ST slot WAR")

            # ---- per-round HWDGE stores for runs f=0..7 (one round deferred;
            # alternating sync/scalar so each ring sees 4 issues/round) ----
            if r >= 1:
                for f in range(HW_MAX_F):
                    eng = nc.scalar if f % 2 == 0 else nc.sync
                    emit_store((r - 1) // SLAB, r - 1, r, f, eng)

            # ---- gpsimd slab stores for all runs; last slab split 2+2
            # rounds so its second half is the only store tail ----
            if rs == SLAB - 1:
                runs = list(range(HW_MAX_F, F - 1))
                if slab < R // SLAB - 1:
                    for i, f in enumerate(runs):
                        tgt = min(r + 1 + (i % SLAB), R)
                        pending.setdefault(tgt, []).append(
                            (slab, slab * SLAB, (slab + 1) * SLAB, f, nc.gpsimd)
                        )
                else:
                    for i, f in enumerate(runs):
                        pending.setdefault(r - 1 + (i % 2), []).append(
                            (slab, r - 3, r - 1, f, nc.gpsimd)
                        )
                    for f in runs:
                        pending.setdefault(R, []).append(
                            (slab, r - 1, r + 1, f, nc.gpsimd)
                        )

        # ---- tail: last round's HWDGE runs + remaining gpsimd slab stores
        for f in range(HW_MAX_F):
            eng = nc.scalar if f % 2 == 0 else nc.sync
            emit_store((R - 1) // SLAB, R - 1, R, f, eng)
        for rr in sorted(pending):
            for args in pending[rr]:
                emit_store(*args)

    nc.finalize()
    return nc


def make_in_maps(dense_output: np.ndarray, embeddings: np.ndarray):
    import ml_dtypes

    dense_output = np.ascontiguousarray(np.asarray(dense_output, dtype=np.float32))
    embeddings = np.ascontiguousarray(np.asarray(embeddings, dtype=np.float32))
    den_bf = np.ascontiguousarray(dense_output.astype(ml_dtypes.bfloat16))
    emb_bf = np.ascontiguousarray(embeddings.astype(ml_dtypes.bfloat16))
    in_maps = []
    for i in range(N_CORES):
        sl = slice(i * BC, (i + 1) * BC)
        in_maps.append(
            {
                "dense_f32": np.ascontiguousarray(dense_output[sl]),
                "dense_bf16": np.ascontiguousarray(den_bf[sl]),
                "emb_bf16": np.ascontiguousarray(emb_bf[sl]),
            }
        )
    return in_maps


def kernel(dense_output: np.ndarray, embeddings: np.ndarray) -> np.ndarray:
    from concourse.bass_utils import run_bass_kernel_spmd

    if "nc" not in _CACHE:
        _CACHE["nc"] = _build_nc()
    nc = _CACHE["nc"]

    in_maps = make_in_maps(dense_output, embeddings)
    res = run_bass_kernel_spmd(nc, in_maps, list(range(N_CORES)))
    return np.concatenate([res.results[i]["out"] for i in range(N_CORES)], axis=0)


# revision 26
# speedup vs baseline: 1.0449x; 1.0449x over previous
"""DLRM DotInteractionArch kernel for 8x Trainium2 NeuronCores.

Problem: B=16384, 26 sparse embeddings + 1 dense feature, D=128.
  combined[b] = concat(dense[b], emb[b])           # [27, 128]
  G[b] = combined[b] @ combined[b].T               # [27, 27]
  out[b] = concat(dense[b], triu(G[b], k=1).flat)  # [479]

V11 strategy (pure data parallel, 2048 samples/core, 16 rounds x 128 samples):
  - The fp32->bf16 cast happens ON THE HOST (numpy/ml_dtypes) inside
    kernel(); the device receives bf16 embeddings + bf16 dense (for the
    interaction matmuls) and an fp32 dense copy (for the exact passthrough
    columns). This halves HBM load bytes and avoids the SWDGE cast path,
    which converts at only ~8GB/s per DMA engine.
  - Loads SAMPLE-MAJOR: one SWDGE DMA per round per tensor
    ([128 b-partitions, 26*128] bf16, 128 fat descriptors, ~6.7KB each).
  - PE transpose pass: 27 bf16 is_transpose matmuls [128b,128d] -> PSUM
    bf16. Evac copy reads PT [d, j, b] with a rearranged (strided) PSUM AP
    and writes CT [128 d, 128 b, 32 f-slot] bf16 contiguously.
    (Reordered DVE *reads* are fine; reordered bf16 *writes* crash.)
  - G-pass: per-sample col-tiled matmuls with CONTIGUOUS operands
    CT[:, b, 0:27] (tile_position (0,32s)) -> PSG [32s+f, q, g'] fp32.
    Evacuated by the Scalar engine (ACT) into 4-round slab staging.
  - Stores: run-f descriptors are partition-pinned to SDMA engine pair
    (f//4)*2,(+1); HWDGE-ring descriptors additionally pin to engines 0-3.
    So runs f=0..7 (engines 0-3 either way) go per-round on sync/scalar
    HWDGE (many small concurrent transfers pipeline best), and runs
    f=8..25 go on gpsimd SWDGE in 4-round slabs (descriptors spread to
    engines 4-13; Q7 emission amortized 4x), emitted spread over the
    rounds after each slab.
  - Dense passthrough: one fp32 HBM->HBM SWDGE DMA.
  - gpsimd slab stores use single_packet=True: their ~512 scattered
    54B descriptors pack into single packets, cutting per-packet
    m2s<->s2m context-switch overhead on the SDMA engines (-6% wall;
    on the HWDGE per-round stores the same flag HURTS - they rely on
    multi-engine spread).

NOTE: stride-partition APs (ST[f::32]) are invisible to the Tile
shadow-memory dependency tracker, so RAW/WAR edges around the store DMAs
are wired explicitly with add_dep_helper.

Sample mapping per core: b = r*128 + g*4 + s  (r: round, g = h*16+q, s: 0..3)
"""

import numpy as np

B_FULL = 16384
N_CORES = 8
BC = B_FULL // N_CORES  # 2048 samples per core
F = 27                  # 1 dense + 26 sparse features
D = 128
NSPARSE = 26
SPR = 128               # samples per round
PAIRS = F * (F - 1) // 2  # 351
OUTC = D + PAIRS          # 479

_CACHE = {}


def _triu_offsets():
    off = [D]
    for f in range(F - 1):
        off.append(off[-1] + (NSPARSE - f))
    return off


def _build_nc(bc: int = BC):
    from contextlib import ExitStack

    import concourse.bacc as bacc
    import concourse.tile as tile
    from concourse import mybir
    from concourse.masks import make_identity
    from concourse.tile_rust import add_dep_helper

    BF = mybir.dt.bfloat16
    F32 = mybir.dt.float32
    R = bc // SPR  # 16 rounds
    SLAB = 4       # staging slab size in rounds
    ST_BUFS = 3
    HW_MAX_F = 8   # runs f < 8 per-round on HWDGE; f >= 8 on gpsimd slabs

    nc = bacc.Bacc("TRN2", target_bir_lowering=False, debug=False)
    den_f = nc.dram_tensor("dense_f32", [bc, D], F32, kind="ExternalInput")
    den = nc.dram_tensor("dense_bf16", [bc, D], BF, kind="ExternalInput")
    emb = nc.dram_tensor("emb_bf16", [bc, NSPARSE, D], BF, kind="ExternalInput")
    out = nc.dram_tensor("out", [bc, OUTC], F32, kind="ExternalOutput")

    off = _triu_offsets()

    emb_v = emb.ap().rearrange("(r p) j d -> r p j d", p=SPR)
    den_v = den.ap().rearrange("(r p) d -> r p d", p=SPR)
    out_v = out.ap().rearrange("(r h q s) c -> r s h q c", h=2, q=16, s=4)

    with tile.TileContext(nc) as tc, ExitStack() as ctx:
        const = ctx.enter_context(tc.tile_pool(name="const", bufs=1))
        xep = ctx.enter_context(tc.tile_pool(name="xe", bufs=3))
        xdp = ctx.enter_context(tc.tile_pool(name="xd", bufs=3))
        ctp = ctx.enter_context(tc.tile_pool(name="ct", bufs=3))
        stp = ctx.enter_context(tc.tile_pool(name="st", bufs=ST_BUFS))
        ptp = ctx.enter_context(tc.tile_pool(name="pt", bufs=3, space="PSUM"))
        psgp = ctx.enter_context(tc.tile_pool(name="psg", bufs=4, space="PSUM"))

        ident = const.tile([128, 128], BF)
        make_identity(nc, ident)

        # dense passthrough columns: one fp32 HBM->HBM DMA
        nc.gpsimd.dma_start(out=out.ap()[:, 0:D], in_=den_f.ap()[:, :])

        st_copies = {}   # slab -> [evac insts]
        slab_dmas = {}   # slab -> [store insts] (for WAR on slot reuse)
        pending = {}     # round -> [(slab, r_lo, r_hi, f, eng)]
        ST_tiles = {}    # slab -> tile

        def emit_store(slab, r_lo, r_hi, f, eng):
            n = NSPARSE - f
            src = ST_tiles[slab][f::32, r_lo - slab * SLAB:r_hi - slab * SLAB,
                                 :, :, f + 1:F]
            dst = (
                out.ap()[r_lo * SPR:r_hi * SPR, off[f]:off[f] + n]
                .rearrange("(r h q s) c -> s r h q c", h=2, q=16, s=4)
            )
            d = eng.dma_start(out=dst, in_=src,
                              single_packet=(eng is nc.gpsimd))
            slab_dmas.setdefault(slab, []).append(d)
            for cpy in st_copies[slab]:
                add_dep_helper(d.ins, cpy.ins, reason="triu DMA RAW on ST")

        for r in range(R):
            slab = r // SLAB

            # ---- stores scheduled for this round ----
            for args in pending.pop(r, []):
                emit_store(*args)

            # ---- loads: sample-major bf16 on the sync HWDGE ring (keeps them
            # out of the gpsimd SWDGE ring, where slab-store descriptor bursts
            # would block them FIFO-style) ----
            XE = xep.tile([128, NSPARSE, D], BF)
            XD = xdp.tile([128, D], BF)
            nc.sync.dma_start(out=XE[:], in_=emb_v[r])
            nc.sync.dma_start(out=XD[:], in_=den_v[r])

            # ---- T-pass: 27 bf16 transposes [128 b, 128 d] -> PT [128 d, j, 128 b]
            CT = ctp.tile([128, 128, 32], BF)  # [d, b, f-slot]
            for t in range(7):
                js = list(range(4 * t, min(4 * t + 4, F)))
                k = len(js)
                PT = ptp.tile([128, 4, 128], BF)
                for i, j in enumerate(js):
                    src = XD[:] if j == 0 else XE[:, j - 1]
                    nc.tensor.transpose(PT[:, i], src, ident[:])
                pin = PT[:, 0:k].rearrange("d j b -> d b j")
                nc.vector.tensor_copy(CT[:, :, js[0]:js[0] + k], pin)

            # ---- G-pass: per-sample col-tiled matmuls -> PSG [32s+f, q, g']
            if slab not in ST_tiles:
                ST_tiles[slab] = stp.tile([128, SLAB, 2, 16, 32], F32, name="ST")
                st_copies[slab] = []
            ST = ST_tiles[slab]
            rs = r % SLAB
            for h in range(2):
                PSG = psgp.tile([128, 16, 32], F32)
                for q in range(16):
                    g = h * 16 + q
                    for s in range(4):
                        c = CT[:, 4 * g + s, 0:F]  # [128 d, 27 f] contiguous
                        nc.tensor.matmul(
                            PSG[32 * s:32 * s + F, q, 0:F],
                            c,
                            c,
                            start=True,
                            stop=True,
                            tile_position=(0, 32 * s),
                        )
                cpy = nc.scalar.copy(ST[:, rs, h], PSG[:])
                st_copies[slab].append(cpy)
                # WAR: this evac reuses the slot read by slab-ST_BUFS stores
                for d in slab_dmas.get(slab - ST_BUFS, []):
                    add_dep_helper(cpy.ins, d.ins, reason="ST slot WAR")

            # ---- per-round HWDGE stores for runs f=0..7 (one round deferred;
            # alternating sync/scalar so each ring sees 4 issues/round) ----
            if r >= 1:
                for f in range(HW_MAX_F):
                    eng = nc.scalar if f % 2 == 0 else nc.sync
                    emit_store((r - 1) // SLAB, r - 1, r, f, eng)

            # ---- gpsimd 4-round slab stores for runs f=8..25 ----
            if rs == SLAB - 1:
                runs = list(range(HW_MAX_F, F - 1))
                for i, f in enumerate(runs):
                    tgt = min(r + 1 + (i % SLAB), R)
                    pending.setdefault(tgt, []).append(
                        (slab, slab * SLAB, (slab + 1) * SLAB, f, nc.gpsimd)
                    )

        # ---- tail: last round's HWDGE runs + remaining gpsimd slab stores
        for f in range(HW_MAX_F):
            eng = nc.scalar if f % 2 == 0 else nc.sync
            emit_store((R - 1) // SLAB, R - 1, R, f, eng)
        for rr in sorted(pending):
            for args in pending[rr]:
                emit_store(*args)

    nc.finalize()
    return nc


def make_in_maps(dense_output: np.ndarray, embeddings: np.ndarray):
    import ml_dtypes

    dense_output = np.ascontiguousarray(np.asarray(dense_output, dtype=np.float32))
    embeddings = np.ascontiguousarray(np.asarray(embeddings, dtype=np.float32))
    den_bf = np.ascontiguousarray(dense_output.astype(ml_dtypes.bfloat16))
    emb_bf = np.ascontiguousarray(embeddings.astype(ml_dtypes.bfloat16))
    in_maps = []
    for i in range(N_CORES):
        sl = slice(i * BC, (i + 1) * BC)
        in_maps.append(
            {
                "dense_f32": np.ascontiguousarray(dense_output[sl]),
                "dense_bf16": np.ascontiguousarray(den_bf[sl]),
                "emb_bf16": np.ascontiguousarray(emb_bf[sl]),
            }
        )
    return in_maps


def kernel(dense_output: np.ndarray, embeddings: np.ndarray) -> np.ndarray:
    from concourse.bass_utils import run_bass_kernel_spmd

    if "nc" not in _CACHE:
        _CACHE["nc"] = _build_nc()
    nc = _CACHE["nc"]

    in_maps = make_in_maps(dense_output, embeddings)
    res = run_bass_kernel_spmd(nc, in_maps, list(range(N_CORES)))
    return np.concatenate([res.results[i]["out"] for i in range(N_CORES)], axis=0)
